# revision 53
# baseline (speedup 1.0000x reference)
"""Supervised-contrastive loss on 8 Trainium2 NeuronCores.

Math (reference):
    z = x / max(||x||, 1e-8)                  row-normalize
    sim = (z @ z.T) / TEMP                    [N, N]
    per-anchor: pos-mean over same-class (excl. self) and logsumexp over
    j != i, then per-class mean, then mean over classes.

exp(sim) is symmetric, so only half the matrix is computed ("wrapped
diagonal band"): anchors are split into 64 chunks of 128 rows; row-chunk
t computes column-chunks d = 0..32 ahead of it (mod 64).  A pair (i, j)
with chunk distance d is computed once (at the nearer row) for
1 <= d <= 31, at both rows for d == 32 -- the d=32 cell's exp carries
bias = -ln2 so each side contributes exactly half.  Row sums over the
band ride on the ScalarE Exp via accum_out; the "missing" transposed
halves are recovered as column sums: each exp tile (bf16, SBUF) is
added by the DVE into a per-core [128, 5120] accumulator, which is
DMA'd out raw and partition-reduced on the host.

Core c owns the CONSECUTIVE row-chunks t = 8c + k (k = 0..7); its z8
copy is column-rotated by 1024*c on the host, so the union of its
bands is rotated cols [0, 5120) -- only 6144 rotated columns of z8 are
shipped (the band never wraps) and the SBUF addresses are identical on
every core (SPMD shares one instruction stream).  Class-segment sums
come from a small GEMM tm = A @ W.T with W[c] = sum of z8 rows of
class c (host-precomputed).  The diagonal sim[i,i] = ||z8[i]||^2 is
reconstructed exactly on host and subtracted there.

Layout: all fp8 operands are host-packed for DoubleRow so that feature
d = kk*256 + i*128 + p lands on partition p, plane i of contraction
tile kk; every DMA is per-partition contiguous (strided DGE issues
cost ~3x on the sync sequencer).

Hardware notes baked into this structure: DMAs only from nc.sync, one
matmul accumulation group per PSUM bank, fp8 DoubleRow streams 1
output element per cycle per 256-deep pass (157 TF/s peak), ScalarE is
1 elem/lane/cycle at 1.2 GHz (the old full-matrix kernel was
bottlenecked on it), DMA moves only ~0.3 MB/us end-to-end (input
volume is minimized and output regions ship as soon as their last
writer retires), and the d=32 runt cells run as a tail phase so the
two rotating [128, 2048] PSUM slots never stall the PE mid-loop.
"""

import math

import numpy as np
import ml_dtypes

N = 8192           # anchors
D = 768            # feature dim
NOP = 64           # number of classes
CORES = 8
KT8 = D // 256     # 3 double-row contraction tiles
NROW = 8           # 128-row chunks per core
CELLW = 2048       # wide cell width (one PSUM slot, 4 banks)
RUNTW = 128        # d=32 runt cell width
ZCOLS = 5120       # rotated z8 columns shipped per core
ACCW = 5120        # rotated colsum extent per core
# z8 DMA group widths (first two small so row 0 starts early)
GWS = [512, 512, 1024, 1024, 1024, 1024]
GSTART = [0, 512, 1024, 2048, 3072, 4096]
NGZ = len(GWS)
TEMP_INV = 10.0
EPS = 1e-8

FP8 = ml_dtypes.float8_e4m3

_CACHE = {}
LAST_RESULT = None  # BassKernelResults of the most recent run (for profiling)


def _build_nc():
    from concourse import bacc
    import concourse.mybir as mybir
    import concourse.tile as tile

    f8 = mybir.dt.float8e4
    f32 = mybir.dt.float32
    bf16 = mybir.dt.bfloat16
    Exp = mybir.ActivationFunctionType.Exp
    DR = mybir.MatmulPerfMode.DoubleRow

    nc = bacc.Bacc(
        "TRN2", target_bir_lowering=False, debug=False, enable_asserts=False
    )
    z8 = nc.dram_tensor("z8", [128, KT8 * 2 * ZCOLS], f8, kind="ExternalInput").ap()
    a8 = nc.dram_tensor("a8", [128, NROW, KT8, 2, 128], f8, kind="ExternalInput").ap()
    w8 = nc.dram_tensor("w8", [128, KT8, 2, NOP], f8, kind="ExternalInput").ap()
    tm = nc.dram_tensor("tm", [128, NROW, NOP], f32, kind="ExternalOutput").ap()
    pacc = nc.dram_tensor("pacc", [128, NROW, 5], f32, kind="ExternalOutput").ap()
    acc_out = nc.dram_tensor("acc_out", [128, ACCW], bf16, kind="ExternalOutput").ap()

    with tile.TileContext(nc) as tc:
        with (
            tc.tile_pool(name="zin", bufs=4) as zin,
            tc.tile_pool(name="epool", bufs=3) as epool,
            tc.tile_pool(name="singles", bufs=1) as singles,
        ):
            # ---- input DMAs (all per-partition contiguous), ordered for
            # earliest row-0 start ----
            a8_sb = singles.tile([128, NROW, KT8, 2, 128], f8)
            nc.sync.dma_start(out=a8_sb[:, :3], in_=a8[:, :3])
            w8_sb = singles.tile([128, KT8, 2, NOP], f8)
            nc.sync.dma_start(out=w8_sb, in_=w8)
            z8_sb = {}

            def dma_z8(g, eng=None):
                gw = GWS[g]
                z8_t = zin.tile(
                    [128, KT8, 2, gw], f8, name="z8_t", tag=f"z8_{gw}",
                    bufs=GWS.count(gw),
                )
                f0 = KT8 * 2 * GSTART[g]
                (eng or nc.sync).dma_start(
                    out=z8_t.rearrange("p a b c -> p (a b c)"),
                    in_=z8[:, f0:f0 + KT8 * 2 * gw],
                )
                z8_sb[g] = z8_t

            dma_z8(0)
            dma_z8(1)
            dma_z8(2, eng=nc.scalar)
            dma_z8(3)
            nc.sync.dma_start(out=a8_sb[:, 3:], in_=a8[:, 3:])
            dma_z8(4, eng=nc.scalar)
            dma_z8(5)

            # colsum accumulator + rowsum slots, zeroed during the DMA fill
            acc = singles.tile([128, ACCW], bf16)
            nc.vector.memset(acc, 0.0)
            pacc_sb = singles.tile([128, NROW, 5], f32)
            nc.vector.memset(pacc_sb.rearrange("p a b -> p (a b)"), 0.0)
            tm_sb = singles.tile([128, NROW, NOP], f32)

            # bias = -ln2 for the d=32 runt cells (halves their exp)
            nln2 = singles.tile([128, 1], f32)
            nc.vector.memset(nln2, -math.log(2.0))

            ps_pool = tc.alloc_tile_pool(name="ps", bufs=2, space="PSUM")

            def do_cell(k, slot, start, w, bias, skip_head):
                """One band cell: sim matmuls -> Exp(+rowsum) -> DVE colsum.

                skip_head: first 128 cols are the d=0 diagonal chunk,
                excluded from the colsum accumulator.
                """
                ps_t = ps_pool.tile([128, w], f32, name="ps_t", tag="ps_t")
                # 512-col slices; a slice crossing a z8-group boundary is
                # split into pieces.  matmul start resets the whole PSUM
                # bank, so only the bank's FIRST matmul carries start=True
                # (the reset zeroes the later pieces' region too).  kk
                # outer so the lhsT weights are reused across the slices.
                slices = []
                for jj in range(0, w, 512):
                    pieces = []
                    o = 0
                    while o < min(512, w - jj):
                        col = start + jj + o
                        g = next(i for i in reversed(range(NGZ))
                                 if GSTART[i] <= col)
                        off = col - GSTART[g]
                        sw = min(512 - o, w - jj - o, GWS[g] - off)
                        pieces.append((jj + o, g, off, sw))
                        o += sw
                    slices.append(pieces)
                for kk in range(KT8):
                    lhsT = a8_sb[:, k, kk]
                    for pieces in slices:
                        for pi, (o, g, off, sw) in enumerate(pieces):
                            nc.tensor.matmul(
                                ps_t[:, o:o + sw],
                                lhsT,
                                z8_sb[g][:, kk, :, off:off + sw],
                                start=(kk == 0 and pi == 0),
                                stop=(kk == KT8 - 1 and pi == len(pieces) - 1),
                                perf_mode=DR,
                                skip_group_check=True,
                            )
                e_t = epool.tile([128, w], bf16, name="e_t", tag="e_t")
                nc.scalar.activation(
                    out=e_t,
                    in_=ps_t,
                    func=Exp,
                    scale=TEMP_INV,
                    bias=bias,
                    accum_out=pacc_sb[:, k, slot:slot + 1],
                )
                eoff = RUNTW if skip_head else 0
                if w > eoff:
                    s0 = start + eoff
                    nc.vector.tensor_add(
                        acc[:, s0:s0 + w - eoff],
                        acc[:, s0:s0 + w - eoff],
                        e_t[:, eoff:w],
                    )

            def do_tm(k, pool, tag):
                pst = pool.tile([128, NOP], f32, name="tm_t", tag=tag)
                for kk in range(KT8):
                    nc.tensor.matmul(
                        pst,
                        a8_sb[:, k, kk],
                        w8_sb[:, kk, :, :],
                        start=(kk == 0),
                        stop=(kk == KT8 - 1),
                        perf_mode=DR,
                    )
                nc.vector.tensor_copy(tm_sb[:, k, :], pst)

            # tm rows 0-2 need only a8 head + w8: free PE work while z8
            # group 0 is still streaming in
            for k in range(3):
                do_tm(k, ps_pool, "ps_t")

            # ---- main band: near cells (d 0..15) for all rows, then far
            # cells (d 16..31); row 0's first cell is split so compute
            # starts after only the first 512 z8 columns have landed.
            # tm rows 3-7 and the batched d=32 runts interleave between
            # far cells (the PE work they add exceeds the ACT latency they
            # hide behind, so no PSUM-slot stalls); the far cells run in
            # order 6,7,5,...,1 with k=0 last, split in half, so the acc
            # output regions finalize in short chains at the very end ----
            do_cell(0, 0, 0, 512, 0.0, True)
            do_cell(0, 1, 512, 1536, 0.0, False)
            for k in range(1, NROW):
                do_cell(k, 0, 128 * k, CELLW, 0.0, True)
                if k == 6:
                    nc.sync.dma_start(out=acc_out[:, :1024], in_=acc[:, :1024])
                if k == 7:
                    nc.sync.dma_start(
                        out=acc_out[:, 1024:2048], in_=acc[:, 1024:2048]
                    )

            for i, k in enumerate([6, 7, 5, 4, 3]):
                do_cell(k, 2, 128 * k + CELLW, CELLW, 0.0, False)
                do_tm(3 + i, ps_pool, "ps_t")
            nc.sync.dma_start(out=tm, in_=tm_sb)
            do_cell(2, 2, 128 * 2 + CELLW, CELLW, 0.0, False)
            do_cell(1, 2, 128 * 1 + CELLW, CELLW, 0.0, False)

            # batched d=32 runt cells (halved via bias=-ln2): all 8 sims
            # in one [128, 1024] PSUM tile (start resets a whole bank, so
            # only each bank's first matmul carries it), one wide Exp,
            # rowsums via one batched DVE reduce, one DVE add (runt k's
            # cols sit exactly at acc col 4096+128k)
            rp = ps_pool.tile([128, NROW * RUNTW], f32, name="ps_t", tag="ps_t")
            for k in range(NROW):
                col = 128 * k + 4096
                g = next(i for i in reversed(range(NGZ)) if GSTART[i] <= col)
                off = col - GSTART[g]
                for kk in range(KT8):
                    nc.tensor.matmul(
                        rp[:, k * RUNTW:(k + 1) * RUNTW],
                        a8_sb[:, k, kk],
                        z8_sb[g][:, kk, :, off:off + RUNTW],
                        start=(kk == 0 and k % 4 == 0),
                        stop=(kk == KT8 - 1 and k % 4 == 3),
                        perf_mode=DR,
                        skip_group_check=True,
                    )
            e_t = epool.tile([128, NROW * RUNTW], bf16, name="e_t", tag="e_t")
            nc.scalar.activation(
                out=e_t, in_=rp, func=Exp, scale=TEMP_INV, bias=nln2
            )
            nc.vector.tensor_reduce(
                pacc_sb[:, :, 3:4],
                e_t.rearrange("p (k r) -> p k r", k=NROW),
                mybir.AxisListType.X,
                mybir.AluOpType.add,
            )
            nc.vector.tensor_add(acc[:, 4096:ACCW], acc[:, 4096:ACCW], e_t)
            nc.sync.dma_start(out=acc_out[:, 4096:ACCW], in_=acc[:, 4096:ACCW])

            # last far cell (k=0) in halves; each half releases an acc
            # output region as soon as its colsum add retires
            do_cell(0, 2, 2048, 1024, 0.0, False)
            nc.sync.dma_start(out=acc_out[:, 2048:3072], in_=acc[:, 2048:3072])
            do_cell(0, 4, 3072, 1024, 0.0, False)
            nc.sync.dma_start(out=acc_out[:, 3072:4096], in_=acc[:, 3072:4096])
            nc.sync.dma_start(out=pacc, in_=pacc_sb)
            ps_pool.release()

    nc.compile()
    return nc


def _get_nc():
    if "nc" not in _CACHE:
        _CACHE["nc"] = _build_nc()
    return _CACHE["nc"]


def _pack_dr(mat_t):
    """[D, cols] -> [128, KT8, 2, cols] with d = kk*256 + i*128 + p."""
    d, cols = mat_t.shape
    return np.ascontiguousarray(
        mat_t.reshape(KT8, 2, 128, cols).transpose(2, 0, 1, 3)
    )


def kernel(x, op_ids, n_op):
    global LAST_RESULT
    from concourse.bass_utils import run_bass_kernel_spmd

    x = np.asarray(x, dtype=np.float32).reshape(-1, D)
    op_ids = np.asarray(op_ids).reshape(-1).astype(np.int64)
    n_op_i = int(np.asarray(n_op))

    # ---- host prep: normalize, quantize, class sums, diagonal ----
    norms = np.sqrt((x.astype(np.float64) ** 2).sum(axis=1))
    norms = np.maximum(norms, EPS).astype(np.float32)
    z = x / norms[:, None]

    z8 = z.astype(FP8)
    z8f = z8.astype(np.float32)

    onehot = np.zeros((N, NOP), np.float32)
    onehot[np.arange(N), op_ids] = 1.0
    W8 = (onehot.T @ z8f).astype(FP8)               # [NOP, D] fp8

    z8_packed = _pack_dr(np.ascontiguousarray(z8.T))          # [128,3,2,N]
    w8_packed = _pack_dr(np.ascontiguousarray(W8.T.astype(FP8)))
    ssq = (z8f.astype(np.float64) ** 2).sum(axis=1)  # = sim[i, i]

    in_maps = []
    for c in range(CORES):
        # rows 1024c..1024c+1023 as [128, NROW, KT8, 2, 128] lhsT blocks
        a8_c = np.ascontiguousarray(
            z8_packed[:, :, :, 1024 * c:1024 * (c + 1)]
            .reshape(128, KT8, 2, NROW, 128)
            .transpose(0, 3, 1, 2, 4)
        )
        # rotated z8 columns [0, ZCOLS) as NGZ contiguous group blocks
        idx = (np.arange(ZCOLS) + 1024 * c) % N
        zrot = z8_packed[:, :, :, idx]            # [128, KT8, 2, ZCOLS]
        z8_c = np.ascontiguousarray(np.concatenate(
            [zrot[:, :, :, s:s + w].reshape(128, -1)
             for s, w in zip(GSTART, GWS)], axis=1))
        in_maps.append({"z8": z8_c, "a8": a8_c, "w8": w8_packed})

    nc = _get_nc()
    res = run_bass_kernel_spmd(nc, in_maps, core_ids=list(range(CORES)))
    LAST_RESULT = res

    # ---- host post: assemble es = rowsums + colsums, finish loss ----
    es = np.zeros(N, np.float64)
    tm_full = np.zeros((N, NOP), np.float64)
    for c in range(CORES):
        r = res.results[c]
        pacc_c = r["pacc"].astype(np.float64)      # [128, NROW, 5]
        tm_c = r["tm"].astype(np.float64)          # [128, NROW, NOP]
        cs = np.zeros(N, np.float64)
        cs[:ACCW] = r["acc_out"].astype(np.float64).sum(axis=0)
        es += np.roll(cs, 1024 * c)                # unrotate
        for k in range(NROW):
            t = 8 * c + k
            rows = slice(128 * t, 128 * t + 128)
            es[rows] += pacc_c[:, k, :].sum(axis=1)
            tm_full[rows] = tm_c[:, k, :]

    lse = np.log(es - np.exp(TEMP_INV * ssq))
    pos_sum = TEMP_INV * (tm_full[np.arange(N), op_ids] - ssq)
    counts = np.bincount(op_ids, minlength=n_op_i).astype(np.float64)
    pos_cnt = counts[op_ids] - 1.0

    loss_i = np.where(pos_cnt > 0, -pos_sum / np.maximum(pos_cnt, 1.0) + lse, 0.0)
    cls_sum = np.bincount(op_ids, weights=loss_i, minlength=n_op_i)
    cls_loss = np.where(counts > 0, cls_sum / np.maximum(counts, 1.0), 0.0)
    return np.float32(cls_loss.mean())


# revision 54
# speedup vs baseline: 1.0280x; 1.0280x over previous
"""Supervised-contrastive loss on 8 Trainium2 NeuronCores.

Math (reference):
    z = x / max(||x||, 1e-8)                  row-normalize
    sim = (z @ z.T) / TEMP                    [N, N]
    per-anchor: pos-mean over same-class (excl. self) and logsumexp over
    j != i, then per-class mean, then mean over classes.

exp(sim) is symmetric, so only half the matrix is computed ("wrapped
diagonal band"): anchors are split into 64 chunks of 128 rows; row-chunk
t computes column-chunks d = 0..32 ahead of it (mod 64).  A pair (i, j)
with chunk distance d is computed once (at the nearer row) for
1 <= d <= 31, at both rows for d == 32 -- the d=32 cell's exp carries
bias = -ln2 so each side contributes exactly half.  Row sums over the
band ride on the ScalarE Exp via accum_out; the "missing" transposed
halves are recovered as column sums: each exp tile (bf16, SBUF) is
added by the DVE into a per-core [128, 5120] accumulator, which is
DMA'd out raw and partition-reduced on the host.

Core c owns the CONSECUTIVE row-chunks t = 8c + k (k = 0..7); its z8
copy is column-rotated by 1024*c on the host, so the union of its
bands is rotated cols [0, 5120) -- only 6144 rotated columns of z8 are
shipped (the band never wraps) and the SBUF addresses are identical on
every core (SPMD shares one instruction stream).  Class-segment sums
come from a small GEMM tm = A @ W.T with W[c] = sum of z8 rows of
class c (host-precomputed).  The diagonal sim[i,i] = ||z8[i]||^2 is
reconstructed exactly on host and subtracted there.

Layout: all fp8 operands are host-packed for DoubleRow so that feature
d = kk*256 + i*128 + p lands on partition p, plane i of contraction
tile kk; every DMA is per-partition contiguous (strided DGE issues
cost ~3x on the sync sequencer).

Hardware notes baked into this structure: DMAs only from nc.sync, one
matmul accumulation group per PSUM bank, fp8 DoubleRow streams 1
output element per cycle per 256-deep pass (157 TF/s peak), ScalarE is
1 elem/lane/cycle at 1.2 GHz (the old full-matrix kernel was
bottlenecked on it), DMA moves only ~0.3 MB/us end-to-end (input
volume is minimized and output regions ship as soon as their last
writer retires), and the d=32 runt cells run as a tail phase so the
two rotating [128, 2048] PSUM slots never stall the PE mid-loop.
"""

import math

import numpy as np
import ml_dtypes

N = 8192           # anchors
D = 768            # feature dim
NOP = 64           # number of classes
CORES = 8
KT8 = D // 256     # 3 double-row contraction tiles
NROW = 8           # 128-row chunks per core
CELLW = 2048       # wide cell width (one PSUM slot, 4 banks)
RUNTW = 128        # d=32 runt cell width
ZCOLS = 5120       # rotated z8 columns shipped per core
ACCW = 5120        # rotated colsum extent per core
# z8 DMA group widths (first two small so row 0 starts early)
GWS = [512, 512, 1024, 1024, 1024, 1024]
GSTART = [0, 512, 1024, 2048, 3072, 4096]
NGZ = len(GWS)
TEMP_INV = 10.0
EPS = 1e-8

FP8 = ml_dtypes.float8_e4m3

_CACHE = {}
LAST_RESULT = None  # BassKernelResults of the most recent run (for profiling)


def _build_nc():
    from concourse import bacc
    import concourse.mybir as mybir
    import concourse.tile as tile

    f8 = mybir.dt.float8e4
    f32 = mybir.dt.float32
    bf16 = mybir.dt.bfloat16
    Exp = mybir.ActivationFunctionType.Exp
    DR = mybir.MatmulPerfMode.DoubleRow

    nc = bacc.Bacc(
        "TRN2", target_bir_lowering=False, debug=False, enable_asserts=False
    )
    z8 = nc.dram_tensor("z8", [128, KT8 * 2 * ZCOLS], f8, kind="ExternalInput").ap()
    a8 = nc.dram_tensor("a8", [128, NROW, KT8, 2, 128], f8, kind="ExternalInput").ap()
    w8 = nc.dram_tensor("w8", [128, KT8, 2, NOP], f8, kind="ExternalInput").ap()
    tm = nc.dram_tensor("tm", [128, NROW, NOP], f32, kind="ExternalOutput").ap()
    pacc = nc.dram_tensor("pacc", [128, NROW, 5], f32, kind="ExternalOutput").ap()
    acc_out = nc.dram_tensor("acc_out", [128, ACCW], bf16, kind="ExternalOutput").ap()

    with tile.TileContext(nc) as tc:
        with (
            tc.tile_pool(name="zin", bufs=4) as zin,
            tc.tile_pool(name="epool", bufs=3) as epool,
            tc.tile_pool(name="singles", bufs=1) as singles,
        ):
            # ---- input DMAs (all per-partition contiguous), ordered for
            # earliest row-0 start ----
            a8_sb = singles.tile([128, NROW, KT8, 2, 128], f8)
            nc.sync.dma_start(out=a8_sb[:, :3], in_=a8[:, :3])
            w8_sb = singles.tile([128, KT8, 2, NOP], f8)
            nc.sync.dma_start(out=w8_sb, in_=w8)
            z8_sb = {}

            def dma_z8(g, eng=None):
                gw = GWS[g]
                z8_t = zin.tile(
                    [128, KT8, 2, gw], f8, name="z8_t", tag=f"z8_{gw}",
                    bufs=GWS.count(gw),
                )
                f0 = KT8 * 2 * GSTART[g]
                (eng or nc.sync).dma_start(
                    out=z8_t.rearrange("p a b c -> p (a b c)"),
                    in_=z8[:, f0:f0 + KT8 * 2 * gw],
                )
                z8_sb[g] = z8_t

            dma_z8(0)
            dma_z8(1)
            dma_z8(2, eng=nc.scalar)
            dma_z8(3)
            nc.sync.dma_start(out=a8_sb[:, 3:], in_=a8[:, 3:])
            dma_z8(4, eng=nc.scalar)
            dma_z8(5)

            # colsum accumulator + rowsum slots, zeroed during the DMA fill
            acc = singles.tile([128, ACCW], bf16)
            nc.vector.memset(acc, 0.0)
            pacc_sb = singles.tile([128, NROW, 5], f32)
            nc.vector.memset(pacc_sb.rearrange("p a b -> p (a b)"), 0.0)
            tm_sb = singles.tile([128, NROW, NOP], f32)

            # bias = -ln2 for the d=32 runt cells (halves their exp)
            nln2 = singles.tile([128, 1], f32)
            nc.vector.memset(nln2, -math.log(2.0))

            ps_pool = tc.alloc_tile_pool(name="ps", bufs=2, space="PSUM")

            def do_cell(k, slot, start, w, bias, skip_head):
                """One band cell: sim matmuls -> Exp(+rowsum) -> DVE colsum.

                skip_head: first 128 cols are the d=0 diagonal chunk,
                excluded from the colsum accumulator.
                """
                ps_t = ps_pool.tile([128, w], f32, name="ps_t", tag="ps_t")
                # 512-col slices; a slice crossing a z8-group boundary is
                # split into pieces.  matmul start resets the whole PSUM
                # bank, so only the bank's FIRST matmul carries start=True
                # (the reset zeroes the later pieces' region too).  kk
                # outer so the lhsT weights are reused across the slices.
                slices = []
                for jj in range(0, w, 512):
                    pieces = []
                    o = 0
                    while o < min(512, w - jj):
                        col = start + jj + o
                        g = next(i for i in reversed(range(NGZ))
                                 if GSTART[i] <= col)
                        off = col - GSTART[g]
                        sw = min(512 - o, w - jj - o, GWS[g] - off)
                        pieces.append((jj + o, g, off, sw))
                        o += sw
                    slices.append(pieces)
                for kk in range(KT8):
                    lhsT = a8_sb[:, k, kk]
                    for pieces in slices:
                        for pi, (o, g, off, sw) in enumerate(pieces):
                            nc.tensor.matmul(
                                ps_t[:, o:o + sw],
                                lhsT,
                                z8_sb[g][:, kk, :, off:off + sw],
                                start=(kk == 0 and pi == 0),
                                stop=(kk == KT8 - 1 and pi == len(pieces) - 1),
                                perf_mode=DR,
                                skip_group_check=True,
                            )
                e_t = epool.tile([128, w], bf16, name="e_t", tag="e_t")
                nc.scalar.activation(
                    out=e_t,
                    in_=ps_t,
                    func=Exp,
                    scale=TEMP_INV,
                    bias=bias,
                    accum_out=pacc_sb[:, k, slot:slot + 1],
                )
                eoff = RUNTW if skip_head else 0
                if w > eoff:
                    s0 = start + eoff
                    nc.vector.tensor_add(
                        acc[:, s0:s0 + w - eoff],
                        acc[:, s0:s0 + w - eoff],
                        e_t[:, eoff:w],
                    )

            def do_tm(k, pool, tag):
                pst = pool.tile([128, NOP], f32, name="tm_t", tag=tag)
                for kk in range(KT8):
                    nc.tensor.matmul(
                        pst,
                        a8_sb[:, k, kk],
                        w8_sb[:, kk, :, :],
                        start=(kk == 0),
                        stop=(kk == KT8 - 1),
                        perf_mode=DR,
                    )
                nc.vector.tensor_copy(tm_sb[:, k, :], pst)

            # tm rows 0-2 need only a8 head + w8: free PE work while z8
            # group 0 is still streaming in
            for k in range(3):
                do_tm(k, ps_pool, "ps_t")

            # ---- main band: near cells (d 0..15) for all rows, then far
            # cells (d 16..31); row 0's first cell is split so compute
            # starts after only the first 512 z8 columns have landed ----
            do_cell(0, 0, 0, 512, 0.0, True)
            do_cell(0, 1, 512, 1536, 0.0, False)
            for k in range(1, NROW):
                do_cell(k, 0, 128 * k, CELLW, 0.0, True)
                if k == 6:
                    nc.sync.dma_start(out=acc_out[:, :1024], in_=acc[:, :1024])
                if k == 7:
                    nc.sync.dma_start(
                        out=acc_out[:, 1024:2048], in_=acc[:, 1024:2048]
                    )
            for k in range(NROW):
                do_cell(k, 2, 128 * k + CELLW, CELLW, 0.0, False)
            nc.sync.dma_start(out=acc_out[:, 2048:4096], in_=acc[:, 2048:4096])

            # ---- d=32 runt cells (halved via bias=-ln2), batched: all 8
            # sims into one [128, 1024] PSUM tile (start resets a whole
            # bank, so only each bank's first matmul carries it), one wide
            # Exp, rowsums via one batched DVE reduce, one DVE add (runt
            # k's cols sit exactly at acc col 4096+128k) ----
            rp = ps_pool.tile([128, NROW * RUNTW], f32, name="ps_t", tag="ps_t")
            for k in range(NROW):
                col = 128 * k + 4096
                g = next(i for i in reversed(range(NGZ)) if GSTART[i] <= col)
                off = col - GSTART[g]
                for kk in range(KT8):
                    nc.tensor.matmul(
                        rp[:, k * RUNTW:(k + 1) * RUNTW],
                        a8_sb[:, k, kk],
                        z8_sb[g][:, kk, :, off:off + RUNTW],
                        start=(kk == 0 and k % 4 == 0),
                        stop=(kk == KT8 - 1 and k % 4 == 3),
                        perf_mode=DR,
                        skip_group_check=True,
                    )
            e_t = epool.tile([128, NROW * RUNTW], bf16, name="e_t", tag="e_t")
            nc.scalar.activation(
                out=e_t, in_=rp, func=Exp, scale=TEMP_INV, bias=nln2
            )
            nc.vector.tensor_reduce(
                pacc_sb[:, :, 3:4],
                e_t.rearrange("p (k r) -> p k r", k=NROW),
                mybir.AxisListType.X,
                mybir.AluOpType.add,
            )
            nc.vector.tensor_add(acc[:, 4096:ACCW], acc[:, 4096:ACCW], e_t)
            nc.sync.dma_start(out=acc_out[:, 4096:ACCW], in_=acc[:, 4096:ACCW])
            nc.sync.dma_start(out=pacc, in_=pacc_sb)

            # ---- tm rows 3-7 (overlap the acc DMA drain) ----
            for k in range(3, NROW):
                do_tm(k, ps_pool, "ps_t")
            nc.sync.dma_start(out=tm, in_=tm_sb)
            ps_pool.release()

    nc.compile()
    return nc


def _get_nc():
    if "nc" not in _CACHE:
        _CACHE["nc"] = _build_nc()
    return _CACHE["nc"]


def _pack_dr(mat_t):
    """[D, cols] -> [128, KT8, 2, cols] with d = kk*256 + i*128 + p."""
    d, cols = mat_t.shape
    return np.ascontiguousarray(
        mat_t.reshape(KT8, 2, 128, cols).transpose(2, 0, 1, 3)
    )


def kernel(x, op_ids, n_op):
    global LAST_RESULT
    from concourse.bass_utils import run_bass_kernel_spmd

    x = np.asarray(x, dtype=np.float32).reshape(-1, D)
    op_ids = np.asarray(op_ids).reshape(-1).astype(np.int64)
    n_op_i = int(np.asarray(n_op))

    # ---- host prep: normalize, quantize, class sums, diagonal ----
    norms = np.sqrt((x.astype(np.float64) ** 2).sum(axis=1))
    norms = np.maximum(norms, EPS).astype(np.float32)
    z = x / norms[:, None]

    z8 = z.astype(FP8)
    z8f = z8.astype(np.float32)

    onehot = np.zeros((N, NOP), np.float32)
    onehot[np.arange(N), op_ids] = 1.0
    W8 = (onehot.T @ z8f).astype(FP8)               # [NOP, D] fp8

    z8_packed = _pack_dr(np.ascontiguousarray(z8.T))          # [128,3,2,N]
    w8_packed = _pack_dr(np.ascontiguousarray(W8.T.astype(FP8)))
    ssq = (z8f.astype(np.float64) ** 2).sum(axis=1)  # = sim[i, i]

    in_maps = []
    for c in range(CORES):
        # rows 1024c..1024c+1023 as [128, NROW, KT8, 2, 128] lhsT blocks
        a8_c = np.ascontiguousarray(
            z8_packed[:, :, :, 1024 * c:1024 * (c + 1)]
            .reshape(128, KT8, 2, NROW, 128)
            .transpose(0, 3, 1, 2, 4)
        )
        # rotated z8 columns [0, ZCOLS) as NGZ contiguous group blocks
        idx = (np.arange(ZCOLS) + 1024 * c) % N
        zrot = z8_packed[:, :, :, idx]            # [128, KT8, 2, ZCOLS]
        z8_c = np.ascontiguousarray(np.concatenate(
            [zrot[:, :, :, s:s + w].reshape(128, -1)
             for s, w in zip(GSTART, GWS)], axis=1))
        in_maps.append({"z8": z8_c, "a8": a8_c, "w8": w8_packed})

    nc = _get_nc()
    res = run_bass_kernel_spmd(nc, in_maps, core_ids=list(range(CORES)))
    LAST_RESULT = res

    # ---- host post: assemble es = rowsums + colsums, finish loss ----
    es = np.zeros(N, np.float64)
    tm_full = np.zeros((N, NOP), np.float64)
    for c in range(CORES):
        r = res.results[c]
        pacc_c = r["pacc"].astype(np.float64)      # [128, NROW, 5]
        tm_c = r["tm"].astype(np.float64)          # [128, NROW, NOP]
        cs = np.zeros(N, np.float64)
        cs[:ACCW] = r["acc_out"].astype(np.float64).sum(axis=0)
        es += np.roll(cs, 1024 * c)                # unrotate
        for k in range(NROW):
            t = 8 * c + k
            rows = slice(128 * t, 128 * t + 128)
            es[rows] += pacc_c[:, k, :].sum(axis=1)
            tm_full[rows] = tm_c[:, k, :]

    lse = np.log(es - np.exp(TEMP_INV * ssq))
    pos_sum = TEMP_INV * (tm_full[np.arange(N), op_ids] - ssq)
    counts = np.bincount(op_ids, minlength=n_op_i).astype(np.float64)
    pos_cnt = counts[op_ids] - 1.0

    loss_i = np.where(pos_cnt > 0, -pos_sum / np.maximum(pos_cnt, 1.0) + lse, 0.0)
    cls_sum = np.bincount(op_ids, weights=loss_i, minlength=n_op_i)
    cls_loss = np.where(counts > 0, cls_sum / np.maximum(counts, 1.0), 0.0)
    return np.float32(cls_loss.mean())
